# revision 1
# baseline (speedup 1.0000x reference)
# Trainium2 Bass kernel for nn_FFF_v2 (fast-feedforward / MoE tree routing).
#
#   lam   = x @ W.T                      [B, 12] router logits
#   branch= lam > 0                      tree descent decisions
#   node  = (2^i - 1) + sum_{j<i} branch_j 2^(i-1-j)
#   out   = sum_i lam_i * Y[node_i]      [B, 4096]
#
# Sharding: data-parallel on batch across 8 cores (1024 rows each); W and Y
# replicated.  Per core:
#   - router logits via PE matmul (fp32r) on host-pre-transposed x
#   - tree-node ids via small exact fp32 matmuls (powers-of-two weights)
#   - levels 0..K_MM-1: scaled-one-hot matmul against SBUF-resident shallow
#     Y rows (PE, fp32r) -- exploits the massive node reuse at shallow levels
#   - levels K_MM..11: dma_gather of Y rows from HBM + per-partition-scalar
#     FMA (scalar_tensor_tensor) on the vector engine
import numpy as np

DEPTH = 12
B = 8192
D = 4096
N_NODES = 4095
NCORES = 8
B_LOC = B // NCORES          # 1024 rows per core

MACRO = 256                  # batch rows per macro tile
SUB = 128                    # rows per subtile (one partition block)
NSUB = MACRO // SUB          # 2
NMACRO = B_LOC // MACRO      # 4

K_MM = 8                     # levels 0..K_MM-1 handled by one-hot matmul
N_SH = 2 ** K_MM - 1         # shallow nodes (255)
NCHUNK = (N_SH + 127) // 128  # 2
N_GL = DEPTH - K_MM          # gather levels (4)

ROUTER_F32R = False           # fp32r for the router matmul
ONEHOT_F32R = False           # fp32r for the one-hot matmul
DEBUG_IDX = False            # emit idx16/idxr debug outputs

_CACHE = {}


def _level_of(n):
    # level i spans nodes [2^i - 1, 2^(i+1) - 1)
    lev = 0
    while n >= 2 ** (lev + 1) - 1:
        lev += 1
    return lev


def _host_consts():
    # powT[j, i] = powmat[i, j] = 2^(i-1-j) for j < i  (lhsT of prefix matmul)
    powT = np.zeros((DEPTH, DEPTH), np.float32)
    for i in range(DEPTH):
        for j in range(i):
            powT[j, i] = float(1 << (i - 1 - j))
    # offs_w[p, l*8+f] = 2^l - 1 (level offset in wrapped (level, slot) layout)
    offs = np.array([(1 << i) - 1 for i in range(DEPTH)], np.float32)
    offs_w = np.broadcast_to(
        np.repeat(offs, SUB // 16)[None, :], (16, DEPTH * (SUB // 16))
    ).copy()
    # bselT[l, c*128+p] = 1 if level(c*128+p) == l else 0   (lhsT of bc matmul)
    bselT = np.zeros((DEPTH, NCHUNK * 128), np.float32)
    # nrel[p, c] = node - (2^level - 1), or -1 for pad positions
    nrel = np.full((128, NCHUNK), -1.0, np.float32)
    for c in range(NCHUNK):
        for p in range(128):
            n = c * 128 + p
            if n < N_SH:
                lev = _level_of(n)
                bselT[lev, c * 128 + p] = 1.0
                nrel[p, c] = float(n - ((1 << lev) - 1))
    # identity for PE transpose of the stacked [24, MACRO] lam/prefix tile
    ident = np.eye(2 * DEPTH, dtype=np.float32)
    return powT, offs_w, bselT, nrel, ident


def _build_program():
    import concourse.bass as bass
    import concourse.bacc as bacc
    import concourse.mybir as mybir
    import concourse.tile as tile
    from contextlib import ExitStack

    dt = mybir.dt
    f32 = dt.float32
    f32r = dt.float32r
    i16 = dt.int16
    Alu = mybir.AluOpType

    nc = bacc.Bacc(trn_type="TRN2", num_swdge_queues=4)

    # fp32r operands must be produced as fp32r end-to-end for walrus'
    # BIR verifier; the bit layout is identical to f32.
    xdt = f32r if ROUTER_F32R else f32
    ydt = f32r if ONEHOT_F32R else f32
    xt_d = nc.dram_tensor("xt", [NMACRO, 128, 32, MACRO], xdt, kind="ExternalInput")
    y_d = nc.dram_tensor("y", [N_NODES, D], ydt, kind="ExternalInput")
    wt_d = nc.dram_tensor("wt", [128, 32, DEPTH], xdt, kind="ExternalInput")
    powt_d = nc.dram_tensor("powt", [DEPTH, DEPTH], f32, kind="ExternalInput")
    offsw_d = nc.dram_tensor(
        "offsw", [16, DEPTH * (SUB // 16)], f32, kind="ExternalInput"
    )
    bselt_d = nc.dram_tensor("bselt", [DEPTH, NCHUNK * 128], f32, kind="ExternalInput")
    nrel_d = nc.dram_tensor("nrel", [128, NCHUNK], f32, kind="ExternalInput")
    ident_d = nc.dram_tensor("ident", [2 * DEPTH, 2 * DEPTH], f32, kind="ExternalInput")
    out_d = nc.dram_tensor("out", [B_LOC, D], f32, kind="ExternalOutput")
    if DEBUG_IDX:
        dbg16_d = nc.dram_tensor(
            "dbg16", [NMACRO * NSUB, 16, DEPTH * (SUB // 16)], i16,
            kind="ExternalOutput",
        )
        dbgr_d = nc.dram_tensor(
            "dbgr", [NMACRO * NSUB, 128, N_GL * (SUB // 16)], i16,
            kind="ExternalOutput",
        )

    with tile.TileContext(nc) as tc, ExitStack() as ctx:
        consts = ctx.enter_context(tc.tile_pool(name="consts", bufs=1))
        xt_p = ctx.enter_context(tc.tile_pool(name="xt", bufs=2))
        small = ctx.enter_context(tc.tile_pool(name="small", bufs=3))
        small4 = ctx.enter_context(tc.tile_pool(name="small4", bufs=6))
        st_p = ctx.enter_context(tc.tile_pool(name="st", bufs=6))
        g_p = ctx.enter_context(tc.tile_pool(name="g", bufs=3))
        out_p = ctx.enter_context(tc.tile_pool(name="outp", bufs=2))
        dram_p = ctx.enter_context(tc.tile_pool(name="idxd", bufs=8, space="DRAM"))
        ps_lam = ctx.enter_context(tc.tile_pool(name="pslam", bufs=1, space="PSUM"))
        ps_pb = ps_lam
        ps_bc = ctx.enter_context(tc.tile_pool(name="psbc", bufs=2, space="PSUM"))
        ps_tp = ctx.enter_context(tc.tile_pool(name="pstp", bufs=2, space="PSUM"))
        ps_out = ctx.enter_context(tc.tile_pool(name="psout", bufs=3, space="PSUM"))

        # ---- critical-path constant: router weights only ----
        wt_sb = consts.tile([128, 32, DEPTH], xdt)
        nc.sync.dma_start(wt_sb[:], wt_d.ap())

        for m in range(NMACRO):
            # ---- load x^T macro tile [128, 32, MACRO] ----
            xt = xt_p.tile([128, 32, MACRO], xdt, tag="xt")
            nc.sync.dma_start(xt[:, :16, :], xt_d.ap()[m][:, :16, :])
            nc.sync.dma_start(xt[:, 16:, :], xt_d.ap()[m][:, 16:, :])

            # ---- router: lam^T [12, MACRO] = W @ x^T ----
            lam_ps = ps_lam.tile([DEPTH, MACRO], f32, tag="lam")
            for c in range(32):
                nc.tensor.matmul(
                    lam_ps[:], wt_sb[:, c, :], xt[:, c, :],
                    start=(c == 0), stop=(c == 31),
                )

            if m == 0:
                # remaining constants, deferred so the first x tile and the
                # router aren't stuck behind ~4.5 MiB of const DMA traffic
                powt_sb = consts.tile([DEPTH, DEPTH], f32)
                nc.sync.dma_start(powt_sb[:], powt_d.ap())
                offsw_sb = consts.tile([16, DEPTH * (SUB // 16)], f32)
                nc.sync.dma_start(offsw_sb[:], offsw_d.ap())
                bselt_sb = consts.tile([DEPTH, NCHUNK * 128], f32)
                nc.sync.dma_start(bselt_sb[:], bselt_d.ap())
                nrel_sb = consts.tile([128, NCHUNK], f32)
                nc.sync.dma_start(nrel_sb[:], nrel_d.ap())
                ident_sb = consts.tile([2 * DEPTH, 2 * DEPTH], f32)
                nc.sync.dma_start(ident_sb[:], ident_d.ap())
                ysh_sb = consts.tile([128, NCHUNK * D], ydt)
                for c in range(NCHUNK):
                    lo = c * 128
                    hi = min(lo + 128, N_NODES)
                    nc.scalar.dma_start(
                        ysh_sb[: hi - lo, c * D : (c + 1) * D], y_d.ap()[lo:hi, :]
                    )

            # branch bits, lam^T and prefix^T in SBUF (partition 0 based)
            branch = small.tile([DEPTH, MACRO], f32, tag="branch")
            nc.vector.tensor_scalar(branch[:], lam_ps[:], 0.0, None, Alu.is_gt)
            lamT = small.tile([DEPTH, MACRO], f32, tag="lamT")
            nc.scalar.copy(lamT[:], lam_ps[:])

            # prefix^T [12, MACRO] = powmat @ branch  (exact fp32)
            pb_ps = ps_pb.tile([DEPTH, MACRO], f32, tag="lam")
            nc.tensor.matmul(pb_ps[:], powt_sb[:], branch[:], start=True, stop=True)
            pfxT = small.tile([DEPTH, MACRO], f32, tag="pfxT")
            nc.scalar.copy(pfxT[:], pb_ps[:])

            # ---- S^T build: one chunk of 128 shallow nodes at a time ----
            st = []
            for c in range(NCHUNK):
                bc_ps = ps_bc.tile([128, 2 * MACRO], f32, tag="bc")
                nc.tensor.matmul(
                    bc_ps[:, :MACRO], bselt_sb[:, c * 128 : (c + 1) * 128],
                    pfxT[:], start=True, stop=True,
                )
                nc.tensor.matmul(
                    bc_ps[:, MACRO:], bselt_sb[:, c * 128 : (c + 1) * 128],
                    lamT[:], start=True, stop=True,
                )
                lbc = small.tile([128, MACRO], f32, tag="lbc")
                nc.scalar.copy(lbc[:], bc_ps[:, MACRO:])
                stc = st_p.tile([128, MACRO], ydt, tag="st")
                nc.vector.scalar_tensor_tensor(
                    stc[:], bc_ps[:, :MACRO], nrel_sb[:, c : c + 1], lbc[:],
                    Alu.is_equal, Alu.mult,
                )
                st.append(stc)

            for s in range(NSUB):
                bsl = slice(s * SUB, (s + 1) * SUB)
                # ---- lam to batch-partition layout ----
                # (plain identity matmul: out = in.T @ I; avoids the PE
                # transpose mode, which corrupts partitions after fp32r MMs)
                tpw = ps_tp.tile([SUB, 128], f32, tag="tpw")
                tp_ps = tpw[:, :DEPTH]
                nc.tensor.matmul(
                    tp_ps, lamT[:, bsl], ident_sb[:DEPTH, :DEPTH],
                    start=True, stop=True,
                )
                lamb = small4.tile([SUB, DEPTH], f32, tag="lamb")
                nc.vector.tensor_copy(lamb[:], tp_ps)

                # ---- node ids in the 16-partition-wrapped (level, slot)
                # layout dma_gather wants, via per-16-column PE transposes ----
                w_ps = tpw[:16, 16 : 16 + (SUB // 16) * DEPTH].rearrange(
                    "p (f l) -> p f l", f=SUB // 16
                )
                for f in range(SUB // 16):
                    nc.tensor.matmul(
                        w_ps[:, f, :],
                        pfxT[:, s * SUB + f * 16 : s * SUB + (f + 1) * 16],
                        ident_sb[:DEPTH, :DEPTH],
                        start=True, stop=True,
                    )
                idx16 = small4.tile([16, DEPTH, SUB // 16], i16, tag="idx16")
                nc.vector.tensor_tensor(
                    idx16[:], w_ps[:].rearrange("p f l -> p l f"), offsw_sb[:],
                    Alu.add,
                )
                # replicate to all 8 Q7 descriptor-gen cores via a DRAM bounce
                idxd = dram_p.tile([16, N_GL * (SUB // 16)], i16, tag="idxd")
                nc.sync.dma_start(
                    idxd[:], idx16[:, K_MM:, :].rearrange("p l f -> p (l f)")
                )
                idxr = small4.tile([128, N_GL, SUB // 16], i16, tag="idxr")
                for gq in range(8):
                    nc.sync.dma_start(
                        idxr[16 * gq : 16 * (gq + 1), :, :].rearrange(
                            "p l f -> p (l f)"
                        ),
                        idxd[:],
                    )
                if DEBUG_IDX:
                    nc.sync.dma_start(dbg16_d.ap()[m * NSUB + s], idx16[:].rearrange("p l f -> p (l f)"))
                    nc.sync.dma_start(dbgr_d.ap()[m * NSUB + s], idxr[:].rearrange("p l f -> p (l f)"))

                # ---- gather deep levels from HBM ----
                gt = []
                for li in range(N_GL):
                    g = g_p.tile([128, 1, D], f32, tag="g")
                    y_ap = y_d.ap() if ydt == f32 else y_d.ap().bitcast(f32)
                    nc.gpsimd.dma_gather(
                        g[:], y_ap, idxr[:, li, :], SUB, SUB, D,
                        queue_num=li % 4,
                    )
                    gt.append(g)

                out_t = out_p.tile([SUB, D], f32, tag="out")
                for q in range(D // 512):
                    qsl = slice(q * 512, (q + 1) * 512)
                    # one-hot matmul: shallow-level contribution
                    po = ps_out.tile([SUB, 512], f32, tag="po")
                    for c in range(NCHUNK):
                        nc.tensor.matmul(
                            po[:], st[c][:, bsl],
                            ysh_sb[:, c * D + q * 512 : c * D + (q + 1) * 512],
                            start=(c == 0), stop=(c == NCHUNK - 1),
                        )
                    # FMA chain: out = po + sum_l lam_l * gathered_l
                    nc.vector.scalar_tensor_tensor(
                        out_t[:, qsl], gt[0][:, 0, qsl], lamb[:, K_MM : K_MM + 1],
                        po[:], Alu.mult, Alu.add,
                    )
                    for li in range(1, N_GL):
                        nc.vector.scalar_tensor_tensor(
                            out_t[:, qsl], gt[li][:, 0, qsl],
                            lamb[:, K_MM + li : K_MM + li + 1],
                            out_t[:, qsl], Alu.mult, Alu.add,
                        )
                nc.scalar.dma_start(out_d.ap()[m * MACRO + s * SUB :][:SUB, :], out_t[:])

    nc.compile()
    return nc


def _patch_walrus_passes():
    # The default walrus pass list in this environment omits
    # lower_custom_kernel, which the Pool custom instructions (dma_gather)
    # need. Inject it in front of codegen.
    import concourse.bass_utils as bu

    if getattr(bu, "_ant_lck_patched", False):
        return
    bu._ant_lck_patched = True
    orig = bu.run_command

    def run_command(argv, **kw):
        if argv and "walrus_driver" in str(argv[0]):
            argv = list(argv)
            for i, a in enumerate(argv):
                if a == "--pass" and "lower_custom_kernel" not in argv[i + 1]:
                    argv[i + 1] = argv[i + 1].replace(
                        "codegen", "lower_custom_kernel,codegen"
                    )
                    break
        return orig(argv, **kw)

    bu.run_command = run_command


def _get_program():
    if "nc" not in _CACHE:
        _CACHE["nc"] = _build_program()
    return _CACHE["nc"]


def _prep_in_maps(x, W, Y):
    powT, offs_w, bselT, nrel, ident = _host_consts()
    Y = np.ascontiguousarray(Y, np.float32)
    wt = np.ascontiguousarray(
        W.T.reshape(32, 128, DEPTH).transpose(1, 0, 2), np.float32
    )
    in_maps = []
    xr = x.reshape(NCORES, B_LOC, D)
    for c in range(NCORES):
        xt = xr[c].T  # [D, B_LOC]
        xtm = np.ascontiguousarray(
            xt.reshape(32, 128, NMACRO, MACRO).transpose(2, 1, 0, 3), np.float32
        )
        in_maps.append(
            {
                "xt": xtm, "y": Y, "wt": wt, "powt": powT, "offsw": offs_w,
                "bselt": bselT, "nrel": nrel, "ident": ident,
            }
        )
    return in_maps


def kernel(x, W, Y, _trace=False):
    from concourse.bass_utils import run_bass_kernel_spmd

    _patch_walrus_passes()

    nc = _get_program()
    in_maps = _prep_in_maps(np.asarray(x), np.asarray(W), np.asarray(Y))
    res = run_bass_kernel_spmd(nc, in_maps, list(range(NCORES)), trace=_trace)
    out = np.concatenate([res.results[c]["out"] for c in range(NCORES)], axis=0)
    if _trace:
        _CACHE["last_result"] = res
    return out



# revision 13
# speedup vs baseline: 1.6635x; 1.6635x over previous
# Trainium2 Bass kernel for nn_FFF_v2 (fast-feedforward / MoE tree routing).
#
#   lam   = x @ W.T                      [B, 12] router logits
#   branch= lam > 0                      tree descent decisions
#   node  = (2^i - 1) + sum_{j<i} branch_j 2^(i-1-j)
#   out   = sum_i lam_i * Y[node_i]      [B, 4096]
#
# Sharding: data-parallel on batch across 8 cores (1024 rows each); W and Y
# replicated.  Per core:
#   - router logits via PE matmul in exact fp32 (branch signs must match the
#     fp32 reference; bf16 here would flip ~300 branches and fail absmax)
#   - tree-node ids via small exact fp32 matmuls (powers-of-two weights)
#   - levels 0..K_MM-1: scaled-one-hot matmul (bf16) against SBUF-resident
#     shallow Y rows -- exploits the massive node reuse at shallow levels
#   - levels K_MM..11: dma_gather of bf16 Y rows from HBM + full-width
#     bf16 FMA (scalar_tensor_tensor, 2x DVE mode) on the vector engine
#   - PSUM->SBUF copies of the one-hot result run on the Activation engine
#   - output written in bf16 (halves write traffic); host upcasts to fp32
import numpy as np
import ml_dtypes

DEPTH = 12
B = 8192
D = 4096
N_NODES = 4095
NCORES = 8
B_LOC = B // NCORES          # 1024 rows per core

MACRO = 256                  # batch rows per macro tile
SUB = 128                    # rows per subtile (one partition block)
NSUB = MACRO // SUB          # 2
NMACRO = B_LOC // MACRO      # 4
NF = MACRO // 16             # 16 16-row wrap slots per macro

K_MM = 9                     # levels 0..K_MM-1 handled by one-hot matmul
N_SH = 2 ** K_MM - 1         # shallow nodes (511)
NCHUNK = (N_SH + 127) // 128  # 4
N_GL = DEPTH - K_MM          # gather levels (3)

_CACHE = {}


def _level_of(n):
    # level i spans nodes [2^i - 1, 2^(i+1) - 1)
    lev = 0
    while n >= 2 ** (lev + 1) - 1:
        lev += 1
    return lev


def _host_consts():
    # powT[j, i] = powmat[i, j] = 2^(i-1-j) for j < i  (lhsT of prefix matmul)
    powT = np.zeros((DEPTH, DEPTH), np.float32)
    for i in range(DEPTH):
        for j in range(i):
            powT[j, i] = float(1 << (i - 1 - j))
    # offs_w[p, l*NF+f] = 2^l - 1 (level offset in wrapped (level, slot) layout)
    offs = np.array([(1 << i) - 1 for i in range(DEPTH)], np.float32)
    offs_w = np.broadcast_to(
        np.repeat(offs, NF)[None, :], (16, DEPTH * NF)
    ).copy()
    # bselT[l, c*128+p] = 1 if level(c*128+p) == l else 0   (lhsT of bc matmul)
    bselT = np.zeros((DEPTH, NCHUNK * 128), np.float32)
    # nrel[p, c] = node - (2^level - 1), or -1 for pad positions
    nrel = np.full((128, NCHUNK), -1.0, np.float32)
    for c in range(NCHUNK):
        for p in range(128):
            n = c * 128 + p
            if n < N_SH:
                lev = _level_of(n)
                bselT[lev, c * 128 + p] = 1.0
                nrel[p, c] = float(n - ((1 << lev) - 1))
    # identity for PE transpose of the [12, *] lam/prefix tiles
    ident = np.eye(DEPTH, dtype=np.float32)
    return powT, offs_w, bselT, nrel, ident


def _build_program():
    import concourse.bass as bass
    import concourse.bacc as bacc
    import concourse.mybir as mybir
    import concourse.tile as tile
    from contextlib import ExitStack

    dt = mybir.dt
    f32 = dt.float32
    bf16 = dt.bfloat16
    i16 = dt.int16
    Alu = mybir.AluOpType

    nc = bacc.Bacc(trn_type="TRN2", num_swdge_queues=4)

    xt_d = nc.dram_tensor("xt", [NMACRO, 128, 32, MACRO], f32, kind="ExternalInput")
    y_d = nc.dram_tensor("y", [N_NODES, D], bf16, kind="ExternalInput")
    # zero-padded shallow Y rows (pad rows past N_SH must be 0.0, not garbage:
    # the one-hot matmul multiplies them by 0 and 0*NaN would poison PSUM)
    ysh_d = nc.dram_tensor("ysh", [NCHUNK * 128, D], bf16, kind="ExternalInput")
    wt_d = nc.dram_tensor("wt", [128, 32, DEPTH], f32, kind="ExternalInput")
    powt_d = nc.dram_tensor("powt", [DEPTH, DEPTH], f32, kind="ExternalInput")
    offsw_d = nc.dram_tensor("offsw", [16, DEPTH * NF], f32, kind="ExternalInput")
    bselt_d = nc.dram_tensor("bselt", [DEPTH, NCHUNK * 128], f32, kind="ExternalInput")
    nrel_d = nc.dram_tensor("nrel", [128, NCHUNK], f32, kind="ExternalInput")
    ident_d = nc.dram_tensor("ident", [DEPTH, DEPTH], f32, kind="ExternalInput")
    out_d = nc.dram_tensor("out", [B_LOC, D], bf16, kind="ExternalOutput")

    with tile.TileContext(nc) as tc, ExitStack() as ctx:
        consts = ctx.enter_context(tc.tile_pool(name="consts", bufs=1))
        xt_p = ctx.enter_context(tc.tile_pool(name="xt", bufs=2))
        small = ctx.enter_context(tc.tile_pool(name="small", bufs=3))
        small4 = ctx.enter_context(tc.tile_pool(name="small4", bufs=4))
        st_p = ctx.enter_context(tc.tile_pool(name="st", bufs=2 * NCHUNK))
        g_p = ctx.enter_context(tc.tile_pool(name="g", bufs=2 * N_GL))
        out_p = ctx.enter_context(tc.tile_pool(name="outp", bufs=2))
        dram_p = ctx.enter_context(tc.tile_pool(name="idxd", bufs=4, space="DRAM"))
        ps_lam = ctx.enter_context(tc.tile_pool(name="pslam", bufs=1, space="PSUM"))
        ps_pb = ps_lam
        ps_bc = ctx.enter_context(tc.tile_pool(name="psbc", bufs=2, space="PSUM"))
        ps_tp = ctx.enter_context(tc.tile_pool(name="pstp", bufs=1, space="PSUM"))
        ps_out = ctx.enter_context(tc.tile_pool(name="psout", bufs=3, space="PSUM"))

        # ---- critical-path constant: router weights only ----
        wt_sb = consts.tile([128, 32, DEPTH], f32)
        nc.sync.dma_start(wt_sb[:], wt_d.ap())

        # shallow Y rows: issued up-front on the (idle-until-gathers) Pool
        # software-DGE queue so they don't starve behind the xt stream on the
        # sync HWDGE queue (observed 58GB/s trickle + 90us gather delay when
        # these sat on the scalar HWDGE queue)
        ysh_sb = consts.tile([128, NCHUNK * D], bf16)
        for c in range(NCHUNK):
            nc.gpsimd.dma_start(
                ysh_sb[:, c * D : (c + 1) * D],
                ysh_d.ap()[c * 128 : (c + 1) * 128, :],
            )

        for m in range(NMACRO):
            # ---- load x^T macro tile [128, 32, MACRO] ----
            xt = xt_p.tile([128, 32, MACRO], f32, tag="xt")
            nc.sync.dma_start(xt[:, :16, :], xt_d.ap()[m][:, :16, :])
            nc.sync.dma_start(xt[:, 16:, :], xt_d.ap()[m][:, 16:, :])

            # ---- router: lam^T [12, MACRO] = W @ x^T ----
            lam_ps = ps_lam.tile([DEPTH, MACRO], f32, tag="lam")
            for c in range(32):
                nc.tensor.matmul(
                    lam_ps[:], wt_sb[:, c, :], xt[:, c, :],
                    start=(c == 0), stop=(c == 31),
                )

            if m == 0:
                # remaining constants, deferred so the first x tile and the
                # router aren't stuck behind the const DMA traffic
                powt_sb = consts.tile([DEPTH, DEPTH], f32)
                nc.sync.dma_start(powt_sb[:], powt_d.ap())
                offsw_sb = consts.tile([16, DEPTH * NF], f32)
                nc.sync.dma_start(offsw_sb[:], offsw_d.ap())
                bselt_sb = consts.tile([DEPTH, NCHUNK * 128], f32)
                nc.sync.dma_start(bselt_sb[:], bselt_d.ap())
                nrel_sb = consts.tile([128, NCHUNK], f32)
                nc.sync.dma_start(nrel_sb[:], nrel_d.ap())
                ident_sb = consts.tile([DEPTH, DEPTH], f32)
                nc.sync.dma_start(ident_sb[:], ident_d.ap())

            # branch bits, lam^T and prefix^T in SBUF (partition 0 based)
            branch = small.tile([DEPTH, MACRO], f32, tag="branch")
            nc.vector.tensor_scalar(branch[:], lam_ps[:], 0.0, None, Alu.is_gt)
            lamT = small.tile([DEPTH, MACRO], f32, tag="lamT")
            nc.scalar.copy(lamT[:], lam_ps[:])

            # prefix^T [12, MACRO] = powmat @ branch  (exact fp32)
            pb_ps = ps_pb.tile([DEPTH, MACRO], f32, tag="lam")
            nc.tensor.matmul(pb_ps[:], powt_sb[:], branch[:], start=True, stop=True)
            pfxT = small.tile([DEPTH, MACRO], f32, tag="pfxT")
            nc.scalar.copy(pfxT[:], pb_ps[:])

            # ---- S^T build: one chunk of 128 shallow nodes at a time ----
            st = []
            for c in range(NCHUNK):
                bc_ps = ps_bc.tile([128, 2 * MACRO], f32, tag="bc")
                nc.tensor.matmul(
                    bc_ps[:, :MACRO], bselt_sb[:, c * 128 : (c + 1) * 128],
                    pfxT[:], start=True, stop=True,
                )
                nc.tensor.matmul(
                    bc_ps[:, MACRO:], bselt_sb[:, c * 128 : (c + 1) * 128],
                    lamT[:], start=True, stop=True,
                )
                lbc = small.tile([128, MACRO], f32, tag="lbc")
                nc.scalar.copy(lbc[:], bc_ps[:, MACRO:])
                stc = st_p.tile([128, MACRO], bf16, tag="st")
                nc.vector.scalar_tensor_tensor(
                    stc[:], bc_ps[:, :MACRO], nrel_sb[:, c : c + 1], lbc[:],
                    Alu.is_equal, Alu.mult,
                )
                st.append(stc)

            # ---- node ids for the whole macro in the 16-partition-wrapped
            # (level, slot) layout dma_gather wants ----
            tpw = ps_tp.tile([16, NF * DEPTH], f32, tag="tpw")
            w_ps = tpw[:].rearrange("p (f l) -> p f l", f=NF)
            for f in range(NF):
                nc.tensor.matmul(
                    w_ps[:, f, :],
                    pfxT[:, f * 16 : (f + 1) * 16],
                    ident_sb[:],
                    start=True, stop=True,
                )
            idx16 = small4.tile([16, DEPTH, NF], i16, tag="idx16")
            nc.vector.tensor_tensor(
                idx16[:], w_ps[:].rearrange("p f l -> p l f"), offsw_sb[:],
                Alu.add,
            )
            # replicate to all 8 Q7 descriptor-gen cores via a DRAM bounce.
            # On the Pool swdge queue: keeps the sync queue a clean xt
            # stream, and FIFO-orders these ahead of the gathers that
            # consume idxr anyway.
            idxd = dram_p.tile([16, N_GL * NF], i16, tag="idxd")
            nc.gpsimd.dma_start(
                idxd[:], idx16[:, K_MM:, :].rearrange("p l f -> p (l f)")
            )
            idxr = small4.tile([128, N_GL, NF], i16, tag="idxr")
            for gq in range(8):
                nc.gpsimd.dma_start(
                    idxr[16 * gq : 16 * (gq + 1), :, :].rearrange(
                        "p l f -> p (l f)"
                    ),
                    idxd[:],
                )

            for s in range(NSUB):
                bsl = slice(s * SUB, (s + 1) * SUB)
                # ---- lam to batch-partition layout ----
                # (plain identity matmul: out = in.T @ I)
                tp2 = ps_tp.tile([SUB, DEPTH], f32, tag="tp2")
                nc.tensor.matmul(
                    tp2[:], lamT[:, bsl], ident_sb[:],
                    start=True, stop=True,
                )
                # bf16 scalars: keeps every FMA operand 2-byte for the 2x
                # DVE mode (an fp32 scalar was observed to block it)
                lamb = small4.tile([SUB, DEPTH], bf16, tag="lamb")
                nc.vector.tensor_copy(lamb[:], tp2[:])

                # ---- gather deep levels from HBM (bf16 rows) ----
                gt = []
                for li in range(N_GL):
                    g = g_p.tile([128, 1, D], bf16, tag="g")
                    nc.gpsimd.dma_gather(
                        g[:], y_d.ap(),
                        idxr[:, li, s * (SUB // 16) : (s + 1) * (SUB // 16)],
                        SUB, SUB, D,
                        queue_num=li % 4,
                    )
                    gt.append(g)

                # ---- shallow-level one-hot matmul + PSUM->SBUF copy ----
                out_t = out_p.tile([SUB, D], bf16, tag="out")
                for q in range(D // 512):
                    qsl = slice(q * 512, (q + 1) * 512)
                    po = ps_out.tile([SUB, 512], f32, tag="po")
                    for c in range(NCHUNK):
                        nc.tensor.matmul(
                            po[:], st[c][:, bsl],
                            ysh_sb[:, c * D + q * 512 : c * D + (q + 1) * 512],
                            start=(c == 0), stop=(c == NCHUNK - 1),
                        )
                    # Activation engine: convert fp32 PSUM -> bf16 SBUF
                    nc.scalar.copy(out_t[:, qsl], po[:])

                # ---- deep levels: full-width bf16 FMA chain on DVE ----
                for li in range(N_GL):
                    nc.vector.scalar_tensor_tensor(
                        out_t[:], gt[li][:, 0, :],
                        lamb[:, K_MM + li : K_MM + li + 1],
                        out_t[:], Alu.mult, Alu.add,
                    )
                nc.scalar.dma_start(out_d.ap()[m * MACRO + s * SUB :][:SUB, :], out_t[:])

    nc.compile()
    return nc


def _patch_walrus_passes():
    # The default walrus pass list in this environment omits
    # lower_custom_kernel, which the Pool custom instructions (dma_gather)
    # need. Inject it in front of codegen.
    import concourse.bass_utils as bu

    if getattr(bu, "_ant_lck_patched", False):
        return
    bu._ant_lck_patched = True
    orig = bu.run_command

    def run_command(argv, **kw):
        if argv and "walrus_driver" in str(argv[0]):
            argv = list(argv)
            for i, a in enumerate(argv):
                if a == "--pass" and "lower_custom_kernel" not in argv[i + 1]:
                    argv[i + 1] = argv[i + 1].replace(
                        "codegen", "lower_custom_kernel,codegen"
                    )
                    break
        return orig(argv, **kw)

    bu.run_command = run_command


def _get_program():
    if "nc" not in _CACHE:
        _CACHE["nc"] = _build_program()
    return _CACHE["nc"]


def _prep_in_maps(x, W, Y):
    powT, offs_w, bselT, nrel, ident = _host_consts()
    Yb = np.ascontiguousarray(Y, np.float32).astype(ml_dtypes.bfloat16)
    ysh = np.zeros((NCHUNK * 128, D), ml_dtypes.bfloat16)
    ysh[:N_SH] = Yb[:N_SH]
    wt = np.ascontiguousarray(
        W.T.reshape(32, 128, DEPTH).transpose(1, 0, 2), np.float32
    )
    in_maps = []
    xr = x.reshape(NCORES, B_LOC, D)
    for c in range(NCORES):
        xt = xr[c].T  # [D, B_LOC]
        xtm = np.ascontiguousarray(
            xt.reshape(32, 128, NMACRO, MACRO).transpose(2, 1, 0, 3), np.float32
        )
        in_maps.append(
            {
                "xt": xtm, "y": Yb, "ysh": ysh, "wt": wt, "powt": powT,
                "offsw": offs_w, "bselt": bselT, "nrel": nrel, "ident": ident,
            }
        )
    return in_maps


def kernel(x, W, Y, _trace=False):
    from concourse.bass_utils import run_bass_kernel_spmd

    _patch_walrus_passes()

    nc = _get_program()
    in_maps = _prep_in_maps(np.asarray(x), np.asarray(W), np.asarray(Y))
    res = run_bass_kernel_spmd(nc, in_maps, list(range(NCORES)), trace=_trace)
    out = np.concatenate(
        [np.asarray(res.results[c]["out"], dtype=np.float32) for c in range(NCORES)],
        axis=0,
    )
    if _trace:
        _CACHE["last_result"] = res
    return out
